# revision 1
# baseline (speedup 1.0000x reference)
"""Trainium2 Bass kernel for nn_AttentionBlock (B=8, LN=2048, IDM=HDM=ODM=1024).

Sharding: data-parallel over batch, one batch element per NeuronCore (8 cores).

Per-core computation (batch element b):
    queries = i @ q ; keys = i @ k                    [ln, hdm]
    scores  = queries @ keys.T                        [ln, ln]
    att     = softmax(scores, axis=-1)
    vls     = i @ v                                   [ln, idm]
    ret     = att @ vls + i
    out     = leaky_relu(ret @ mlp, 0.2) + bias

Precision strategy: the softmax exponent amplifies matmul operand
rounding, so the Q/K path (q/k projections and scores) uses 3-pass
bf16 split matmuls (hi/lo decomposition, ~fp32 quality). The value/MLP
path tolerates bf16. All accumulation is fp32 in PSUM.

Layout strategy: everything is computed with the contraction dim on
partitions. The host pre-transposes i (iT = i.T) and pre-splits
operands into bf16 hi/lo pairs; on-chip phases:
  A) kT/qT/vls projections (qT, vls staged via DRAM),
  B) per 128-row s-tile: scores -> softmax -> DMA-transpose(att) ->
     att @ vls (+residual) -> @ mlp -> leaky-relu + bias.
"""
import os
import numpy as np
import ml_dtypes

import concourse.bacc as bacc
import concourse.mybir as mybir
import concourse.tile as tile
from concourse import bass_utils

F32 = mybir.dt.float32
BF16 = mybir.dt.bfloat16
Act = mybir.ActivationFunctionType
Axis = mybir.AxisListType

LN = 2048      # sequence length
D = 1024       # idm = hdm = odm
N_CORES = 8
DC = D // 128      # 8 contraction chunks
ST = LN // 128     # 16 s-tiles
TB = LN // 512     # 4 t-blocks (N=512)
NEG_SLOPE = 0.2

_cached_nc = None


def _build():
    nc = bacc.Bacc("TRN2", target_bir_lowering=False, debug=False)

    # Inputs (per core): host provides iT (= i_b.T) and all weights as
    # bf16 hi/lo splits. bias stays fp32.
    iTh = nc.dram_tensor("iTh", [D, LN], BF16, kind="ExternalInput")
    iTl = nc.dram_tensor("iTl", [D, LN], BF16, kind="ExternalInput")
    qh = nc.dram_tensor("qh", [D, D], BF16, kind="ExternalInput")
    ql = nc.dram_tensor("ql", [D, D], BF16, kind="ExternalInput")
    kh = nc.dram_tensor("kh", [D, D], BF16, kind="ExternalInput")
    kl = nc.dram_tensor("kl", [D, D], BF16, kind="ExternalInput")
    vh = nc.dram_tensor("vh", [D, D], BF16, kind="ExternalInput")
    vl = nc.dram_tensor("vl", [D, D], BF16, kind="ExternalInput")
    mlpb = nc.dram_tensor("mlpb", [D, D], BF16, kind="ExternalInput")
    bias = nc.dram_tensor("bias", [LN, D], F32, kind="ExternalInput")
    out_d = nc.dram_tensor("out", [LN, D], F32, kind="ExternalOutput")

    # [D, X] viewed as [128 partitions, DC chunks, X]
    def pcv(t, x):
        return t.ap().rearrange("(c p) x -> p c x", p=128)

    iThv, iTlv = pcv(iTh, LN), pcv(iTl, LN)

    with tile.TileContext(nc) as tc:
        # --- persistent pool (lives through both phases) ---
        with tc.tile_pool(name="pers", bufs=1) as pers, \
             tc.tile_pool(name="dram", bufs=1, space="DRAM") as dram:
            kTh_sb = pers.tile([128, DC, LN], BF16)   # 32 KB/part
            kTl_sb = pers.tile([128, DC, LN], BF16)   # 32 KB/part
            alpha_ap = pers.tile([128, 1], F32)
            nc.vector.memset(alpha_ap, NEG_SLOPE)

            qTh_d = dram.tile([ST, 128, DC, 128], BF16)
            qTl_d = dram.tile([ST, 128, DC, 128], BF16)
            vls_sb = pers.tile([128, ST, D], BF16)   # 32 KB/part

            _psum_cm = tc.tile_pool(name="psum", bufs=1, space="PSUM")
            psum_pool = _psum_cm.__enter__()
            _ps_ctr = [0]

            def prep_psum(name):
                i_ = _ps_ctr[0] % 4
                _ps_ctr[0] += 1
                return psum_pool.tile([128, 512], F32, name=f"{name}{_ps_ctr[0]}", tag=f"sc{i_}")

            # ================= Phase A: projections =================
            with tc.tile_pool(name="pa_it", bufs=1) as pa_it, \
                 tc.tile_pool(name="pa_w", bufs=1) as pa_w, \
                 tc.tile_pool(name="pa_ev", bufs=1) as pa_ev:
                ith_sb = pa_it.tile([128, DC, LN], BF16)
                itl_sb = pa_it.tile([128, DC, LN], BF16)

                def load_w(hi_t, lo_t, chunked=False):
                    wh_sb = pa_w.tile([128, DC, D], BF16, name="wh_sb", tag="wh")
                    wl_sb = pa_w.tile([128, DC, D], BF16, name="wl_sb", tag="wl")
                    if chunked:
                        for dc in range(DC):
                            nc.sync.dma_start(out=wh_sb[:, dc], in_=pcv(hi_t, D)[:, dc])
                            nc.sync.dma_start(out=wl_sb[:, dc], in_=pcv(lo_t, D)[:, dc])
                    else:
                        nc.sync.dma_start(out=wh_sb, in_=pcv(hi_t, D))
                        nc.sync.dma_start(out=wl_sb, in_=pcv(lo_t, D))
                    return wh_sb, wl_sb

                # interleave per-dc chunks so dc=0 deps resolve early
                wq_h = pa_w.tile([128, DC, D], BF16, name="wh_sb", tag="wh")
                wq_l = pa_w.tile([128, DC, D], BF16, name="wl_sb", tag="wl")
                for dc in range(DC):
                    nc.sync.dma_start(out=wq_h[:, dc], in_=pcv(qh, D)[:, dc])
                    nc.sync.dma_start(out=ith_sb[:, dc], in_=iThv[:, dc])
                    nc.sync.dma_start(out=wq_l[:, dc], in_=pcv(ql, D)[:, dc])
                    nc.sync.dma_start(out=itl_sb[:, dc], in_=iTlv[:, dc])
                _wq = (wq_h, wq_l)

                # --- qT[h, s] (3-pass split) -> DRAM hi/lo ---
                wh_sb, wl_sb = _wq
                for hc in range(DC):
                    evh = pa_ev.tile([128, TB, 512], BF16, name="evh", tag="evh")
                    evl = pa_ev.tile([128, TB, 512], BF16, name="evl", tag="evl")
                    for sb in range(TB):
                        ps = prep_psum("ps_prep")
                        s_sl = slice(sb * 512, sb * 512 + 512)
                        for dc in range(DC):
                            first = dc == 0
                            last = dc == DC - 1
                            lw = wh_sb[:, dc, hc * 128:hc * 128 + 128]
                            ll = wl_sb[:, dc, hc * 128:hc * 128 + 128]
                            nc.tensor.matmul(ps, lw, ith_sb[:, dc, s_sl], start=first, stop=False)
                            nc.tensor.matmul(ps, lw, itl_sb[:, dc, s_sl], start=False, stop=False)
                            nc.tensor.matmul(ps, ll, ith_sb[:, dc, s_sl], start=False, stop=last)
                        nc.vector.tensor_copy(evh[:, sb], ps)
                        nc.vector.tensor_sub(evl[:, sb], ps, evh[:, sb])
                    evh_v = evh.rearrange("p b (si x) -> p (b si) x", x=128)
                    evl_v = evl.rearrange("p b (si x) -> p (b si) x", x=128)
                    qTh_dv = qTh_d.rearrange("si p c x -> p si c x")[:, :, hc, :]
                    qTl_dv = qTl_d.rearrange("si p c x -> p si c x")[:, :, hc, :]
                    nc.sync.dma_start(out=qTh_dv, in_=evh_v)
                    nc.sync.dma_start(out=qTl_dv, in_=evl_v)

                # --- kT[h, t] (3-pass split) -> resident SBUF hi/lo ---
                wh_sb, wl_sb = load_w(kh, kl, chunked=True)
                for hc in range(DC):
                    for tb in range(TB):
                        ps = prep_psum("ps_prep3")
                        t_sl = slice(tb * 512, tb * 512 + 512)
                        for dc in range(DC):
                            first = dc == 0
                            last = dc == DC - 1
                            lw = wh_sb[:, dc, hc * 128:hc * 128 + 128]
                            ll = wl_sb[:, dc, hc * 128:hc * 128 + 128]
                            nc.tensor.matmul(ps, lw, ith_sb[:, dc, t_sl], start=first, stop=False)
                            nc.tensor.matmul(ps, lw, itl_sb[:, dc, t_sl], start=False, stop=False)
                            nc.tensor.matmul(ps, ll, ith_sb[:, dc, t_sl], start=False, stop=last)
                        nc.vector.tensor_copy(kTh_sb[:, hc, t_sl], ps)
                        nc.vector.tensor_sub(kTl_sb[:, hc, t_sl], ps, kTh_sb[:, hc, t_sl])

# --- vls[t, e] (2-pass: iT full x vh, iTh x vl) -> DRAM ---
                wh_sb, wl_sb = load_w(vh, vl, chunked=True)
                for tc_ in range(ST):
                    t_sl = slice(tc_ * 128, tc_ * 128 + 128)
                    for eb in range(2):
                        ps = prep_psum("ps_prep2")
                        e_sl = slice(eb * 512, eb * 512 + 512)
                        for dc in range(DC):
                            nc.tensor.matmul(ps, ith_sb[:, dc, t_sl], wh_sb[:, dc, e_sl],
                                             start=(dc == 0), stop=(dc == DC - 1))
                        nc.vector.tensor_copy(vls_sb[:, tc_, e_sl], ps)

                            # ================= Phase B: attention + MLP =================
            with tc.tile_pool(name="pb_big", bufs=1) as pb_big, \
                 tc.tile_pool(name="pb_str", bufs=2) as pb_str, \
                 tc.tile_pool(name="pb_att", bufs=2) as pb_att, \
                 tc.tile_pool(name="pb_one", bufs=1) as pb_one, \
                 tc.tile_pool(name="pb_st", bufs=2) as pb_st:
                pb_sc = psum_pool
                pb_mm = psum_pool
                mlp_sb = pb_big.tile([128, DC, D], BF16)    # 16 KB/part
                mlp_loaded = [False]

                for g in range(4):        # s-groups of 512
                    gs = slice(g * 512, g * 512 + 512)
                    attT_t = pb_one.tile([128, ST, 512], BF16, name="attT", tag="attT")   # 16 KB
                    itg_t = pb_one.tile([128, DC, 512], BF16, name="itg", tag="itg")      # 8 KB
                    ret_t = pb_one.tile([128, DC, 512], BF16, name="ret", tag="ret", bufs=2)  # 8 KB x2

                    for st4 in range(4):
                        si = g * 4 + st4
                        s_sl = slice(si * 128, si * 128 + 128)
                        qtsh = pb_str.tile([128, DC, 128], BF16, name="qtsh", tag="qtsh")
                        qtsl = pb_str.tile([128, DC, 128], BF16, name="qtsl", tag="qtsl")
                        nc.sync.dma_start(out=qtsh, in_=qTh_d[si])
                        nc.sync.dma_start(out=qtsl, in_=qTl_d[si])

                        scs = [
                            pb_sc.tile([128, 512], F32, name=f"sc{tb}", tag=f"sc{tb}")
                            for tb in range(TB)
                        ]
                        for hc in range(DC):
                            first = hc == 0
                            last = hc == DC - 1
                            for tb in range(TB):
                                t_sl = slice(tb * 512, tb * 512 + 512)
                                nc.tensor.matmul(scs[tb], qtsh[:, hc], kTh_sb[:, hc, t_sl], start=first, stop=False)
                                nc.tensor.matmul(scs[tb], qtsh[:, hc], kTl_sb[:, hc, t_sl], start=False, stop=False)
                                nc.tensor.matmul(scs[tb], qtsl[:, hc], kTh_sb[:, hc, t_sl], start=False, stop=last)

                        # Per-t-block softmax: local max + exp immediately
                        # (frees each PSUM bank early), then algebraic
                        # rescale by f_tb = e^(m_tb - M) / S.
                        st_t = pb_st.tile([128, 24], F32, name="st_t", tag="stats")
                        negm4 = st_t[:, 0:4]
                        sums = st_t[:, 4:8]
                        negM = st_t[:, 8:9]
                        S = st_t[:, 9:10]
                        recip = st_t[:, 10:11]
                        g4 = st_t[:, 12:16]
                        f4 = st_t[:, 16:20]
                        gs4 = st_t[:, 20:24]
                        att32 = pb_att.tile([128, LN], F32, name="att32", tag="att32", bufs=1)
                        for tb in range(TB):
                            nc.vector.reduce_max(negm4[:, tb:tb + 1], scs[tb], axis=Axis.X, negate=True)
                            nc.scalar.activation(
                                out=att32[:, tb * 512:tb * 512 + 512], in_=scs[tb],
                                func=Act.Exp, bias=negm4[:, tb:tb + 1], scale=1.0,
                                accum_out=sums[:, tb:tb + 1],
                            )
                        nc.vector.tensor_reduce(negM, negm4, axis=Axis.X, op=mybir.AluOpType.min)
                        nc.scalar.activation(out=g4, in_=negm4, func=Act.Exp, bias=negM, scale=-1.0)
                        nc.vector.tensor_mul(gs4, g4, sums)
                        nc.vector.reduce_sum(S, gs4, axis=Axis.X)
                        nc.vector.reciprocal(recip, S)
                        nc.vector.tensor_scalar_mul(f4, g4, recip)

                        att_t = pb_att.tile([128, LN], BF16, name="att_t", tag="att")
                        for tb in range(TB):
                            nc.vector.tensor_scalar_mul(
                                att_t[:, tb * 512:tb * 512 + 512],
                                att32[:, tb * 512:tb * 512 + 512],
                                f4[:, tb:tb + 1],
                            )
                        nc.sync.dma_start_transpose(
                            out=attT_t[:, :, st4 * 128:st4 * 128 + 128], in_=att_t
                        )

                    # att @ vls (+ residual) -> retT[e, s-block]
                    nc.sync.dma_start(out=itg_t, in_=iThv[:, :, gs])
                    if not mlp_loaded[0]:
                        nc.sync.dma_start(out=mlp_sb, in_=pcv(mlpb, D))
                        mlp_loaded[0] = True
                    for ec in range(DC):
                        psa = pb_mm.tile([128, 512], F32, name="psa", tag="av", bufs=2)
                        for tc_ in range(ST):
                            nc.tensor.matmul(
                                psa,
                                vls_sb[:, tc_, ec * 128:ec * 128 + 128],
                                attT_t[:, tc_, :],
                                start=(tc_ == 0), stop=(tc_ == ST - 1),
                            )
                        nc.vector.tensor_add(ret_t[:, ec, :], psa, itg_t[:, ec, :])

                    # (ret @ mlp) -> leaky relu -> + bias -> out
                    for st4 in range(4):
                        si = g * 4 + st4
                        s_sl = slice(si * 128, si * 128 + 128)
                        bias_t = pb_str.tile([128, D], F32, name="bias_t", tag="bias")
                        nc.gpsimd.dma_start(out=bias_t, in_=bias.ap()[s_sl, :])
                        out_t = pb_str.tile([128, D], F32, name="out_t", tag="out")
                        for ob in range(2):
                            pso = pb_mm.tile([128, 512], F32, name="pso", tag="om", bufs=2)
                            o_sl = slice(ob * 512, ob * 512 + 512)
                            for ec in range(DC):
                                nc.tensor.matmul(
                                    pso,
                                    ret_t[:, ec, st4 * 128:st4 * 128 + 128],
                                    mlp_sb[:, ec, o_sl],
                                    start=(ec == 0), stop=(ec == DC - 1),
                                )
                            nc.scalar.activation(
                                out=out_t[:, o_sl], in_=pso, func=Act.Prelu,
                                bias=0.0, scale=1.0, alpha=alpha_ap,
                            )
                        nc.vector.tensor_add(out_t, out_t, bias_t)
                        nc.gpsimd.dma_start(out=out_d.ap()[s_sl, :], in_=out_t)

            _psum_cm.__exit__(None, None, None)

    nc.compile()
    return nc


def _get_nc():
    global _cached_nc
    if _cached_nc is None:
        _cached_nc = _build()
    return _cached_nc


def _split_bf16(x):
    hi = x.astype(ml_dtypes.bfloat16)
    lo = (x - hi.astype(np.float32)).astype(ml_dtypes.bfloat16)
    return hi, lo


def kernel(i, k, q, v, mlp, bias):
    i = np.asarray(i, dtype=np.float32)
    k = np.asarray(k, dtype=np.float32)
    q = np.asarray(q, dtype=np.float32)
    v = np.asarray(v, dtype=np.float32)
    mlp = np.asarray(mlp, dtype=np.float32)
    bias = np.asarray(bias, dtype=np.float32)

    qh, ql = _split_bf16(q)
    kh, kl = _split_bf16(k)
    vh, vl = _split_bf16(v)
    mlpb = mlp.astype(ml_dtypes.bfloat16)

    shared = dict(qh=qh, ql=ql, kh=kh, kl=kl, vh=vh, vl=vl, mlpb=mlpb, bias=bias)
    in_maps = []
    for b in range(N_CORES):
        iT = np.ascontiguousarray(i[b].T)
        iTh, iTl = _split_bf16(iT)
        in_maps.append(dict(iTh=iTh, iTl=iTl, **shared))

    nc = _get_nc()
    res = bass_utils.run_bass_kernel_spmd(nc, in_maps, core_ids=list(range(N_CORES)))
    return np.stack([res.results[b]["out"] for b in range(N_CORES)])



# revision 4
# speedup vs baseline: 1.5469x; 1.5469x over previous
"""Trainium2 Bass kernel for nn_AttentionBlock (B=8, LN=2048, IDM=HDM=ODM=1024).

Sharding: data-parallel over batch, one batch element per NeuronCore (8 cores).

Math restructure (host precompute, fp64):
    W  = q @ k.T        so scores = (i@q) @ (i@k).T = i @ W @ i.T
    V2 = v @ mlp        so out    = lrelu(att @ (i@V2) + i@mlp) + bias
Per-core on-chip:
    A = i @ W           [ln, idm]   (transposed tiles: AT[e, s])
    scores = A @ i.T    [ln, ln]
    att = softmax(scores)
    U = i @ V2 ; M = i @ mlp
    out = lrelu(att @ U + M) + bias

Precision: the softmax amplifies score errors, so the QK path uses fp16
hi/lo pieces: main passes in fp16 (11-bit mantissa) plus the two cross
terms per matmul packed into fp8-e5m2 DoubleRow instructions (two K=128
contractions per instruction, ~2x rate). The value path (U/M/att@U) is
fp16 single-pass. All accumulation fp32 in PSUM.

Layout: contraction dim always on partitions. iT = i.T pieces come from
the host (ih fp16, i8 = [e5m2(i), e5m2(i - f16(i))] pair). A pieces are
derived on-chip from PSUM (Ah fp16 + A8 e5m2 pair). M is staged via DRAM
and re-added into the att@U PSUM group with an identity matmul.
"""
import numpy as np
import ml_dtypes

import concourse.bacc as bacc
import concourse.mybir as mybir
import concourse.tile as tile
from concourse import bass_utils

F32 = mybir.dt.float32
F16 = mybir.dt.float16
F8E5 = mybir.dt.float8e5
DR = mybir.MatmulPerfMode.DoubleRow
Act = mybir.ActivationFunctionType
Axis = mybir.AxisListType

LN = 2048      # sequence length
D = 1024       # idm = hdm = odm
N_CORES = 8
DC = D // 128      # 8 contraction chunks
ST = LN // 128     # 16 s-tiles
TB = LN // 512     # 4 t-blocks (N=512)
OB = D // 512      # 2 o-blocks
NEG_SLOPE = 0.2

_cached_nc = None


def _build():
    nc = bacc.Bacc("TRN2", target_bir_lowering=False, debug=False)

    ih = nc.dram_tensor("ih", [D, LN], F16, kind="ExternalInput")
    i8 = nc.dram_tensor("i8", [D, 2, LN], F8E5, kind="ExternalInput")
    wh = nc.dram_tensor("wh", [D, D], F16, kind="ExternalInput")
    w8 = nc.dram_tensor("w8", [D, 2, D], F8E5, kind="ExternalInput")
    v2h = nc.dram_tensor("v2h", [D, D], F16, kind="ExternalInput")
    mlph = nc.dram_tensor("mlph", [D, D], F16, kind="ExternalInput")
    bias = nc.dram_tensor("bias", [LN, D], F32, kind="ExternalInput")
    ident = nc.dram_tensor("ident", [128, 128], F16, kind="ExternalInput")
    out_d = nc.dram_tensor("out", [LN, D], F32, kind="ExternalOutput")

    # [D, X] viewed as [128 partitions, DC chunks, X]
    def pcv(t, x):
        return t.ap().rearrange("(c p) x -> p c x", p=128)

    def pcv2(t, x):  # [D, 2, X] -> [p, c, 2, X]
        return t.ap().rearrange("(c p) two x -> p c two x", p=128)

    ihv = pcv(ih, LN)
    i8v = pcv2(i8, LN)

    with tile.TileContext(nc) as tc:
        with tc.tile_pool(name="pers", bufs=1) as pers, \
             tc.tile_pool(name="dram", bufs=1, space="DRAM") as dram:
            ih_sb = pers.tile([128, DC, LN], F16)        # 32 KB/part
            i8_sb = pers.tile([128, DC, 2, LN], F8E5)    # 32 KB/part
            Ah_sb = pers.tile([128, DC, LN], F16)        # 32 KB/part
            A8_sb = pers.tile([128, DC, 2, LN], F8E5)    # 32 KB/part
            U_sb = pers.tile([128, ST, D], F16)          # 32 KB/part
            id_sb = pers.tile([128, 128], F16)
            alpha_ap = pers.tile([128, 1], F32)
            nc.vector.memset(alpha_ap, NEG_SLOPE)
            nc.sync.dma_start(out=id_sb, in_=ident.ap())

            M_d = dram.tile([ST, 128, D], F16)

            _psum_cm = tc.tile_pool(name="psum", bufs=1, space="PSUM")
            psum_pool = _psum_cm.__enter__()
            _ps_ctr = [0]

            def prep_psum(name):
                i_ = _ps_ctr[0] % 4
                _ps_ctr[0] += 1
                return psum_pool.tile([128, 512], F32, name=f"{name}{_ps_ctr[0]}",
                                      tag=f"sc{i_}")

            # ================= Phase A =================
            with tc.tile_pool(name="pa_w", bufs=1) as pa_w:
                # stream inputs chunk-wise so dc=0 deps resolve early
                wh_sb = pa_w.tile([128, DC, D], F16, name="wh_sb", tag="pa1")
                w8_sb = pa_w.tile([128, DC, 2, D], F8E5, name="w8_sb", tag="pa2")
                for dc in range(DC):
                    nc.sync.dma_start(out=wh_sb[:, dc], in_=pcv(wh, D)[:, dc])
                    nc.sync.dma_start(out=ih_sb[:, dc], in_=ihv[:, dc])
                    nc.sync.dma_start(out=w8_sb[:, dc], in_=pcv2(w8, D)[:, dc])
                    nc.sync.dma_start(out=i8_sb[:, dc], in_=i8v[:, dc])

                # --- A = i @ W  ->  AT[e, s] tiles, split to fp16 + e5m2 pair
                for ec in range(DC):
                    e_sl = slice(ec * 128, ec * 128 + 128)
                    for sb_ in range(TB):
                        ps = prep_psum("pa")
                        s_sl = slice(sb_ * 512, sb_ * 512 + 512)
                        for dc in range(DC):
                            nc.tensor.matmul(ps, wh_sb[:, dc, e_sl],
                                             ih_sb[:, dc, s_sl],
                                             start=(dc == 0), stop=False)
                        for dc in range(DC):
                            nc.tensor.matmul(ps, w8_sb[:, dc, :, e_sl],
                                             i8_sb[:, dc, :, s_sl],
                                             start=False, stop=(dc == DC - 1),
                                             perf_mode=DR)
                        nc.vector.tensor_copy(Ah_sb[:, ec, s_sl], ps)
                        nc.vector.tensor_sub(A8_sb[:, ec, 0, s_sl], ps,
                                             Ah_sb[:, ec, s_sl])
                        nc.vector.tensor_copy(A8_sb[:, ec, 1, s_sl], ps)

                # --- U = i @ V2 -> [t, o] fp16 (stationary ih chunks)
                v2_sb = pa_w.tile([128, DC, D], F16, name="v2_sb", tag="pa1")
                for dc in range(DC):
                    nc.sync.dma_start(out=v2_sb[:, dc], in_=pcv(v2h, D)[:, dc])
                for tc_ in range(ST):
                    t_sl = slice(tc_ * 128, tc_ * 128 + 128)
                    for ob in range(OB):
                        ps = prep_psum("pu")
                        o_sl = slice(ob * 512, ob * 512 + 512)
                        for dc in range(DC):
                            nc.tensor.matmul(ps, ih_sb[:, dc, t_sl],
                                             v2_sb[:, dc, o_sl],
                                             start=(dc == 0), stop=(dc == DC - 1))
                        nc.vector.tensor_copy(U_sb[:, tc_, o_sl], ps)

                # --- M = i @ mlp -> [s, o] fp16 -> DRAM staging
                mlp_sb = pa_w.tile([128, DC, D], F16, name="mlp_sb", tag="pa2")
                for dc in range(DC):
                    nc.sync.dma_start(out=mlp_sb[:, dc], in_=pcv(mlph, D)[:, dc])
                with tc.tile_pool(name="pa_m", bufs=2) as pa_m:
                    for tc_ in range(ST):
                        t_sl = slice(tc_ * 128, tc_ * 128 + 128)
                        m_t = pa_m.tile([128, D], F16, name="m_t", tag="mst")
                        for ob in range(OB):
                            ps = prep_psum("pm")
                            o_sl = slice(ob * 512, ob * 512 + 512)
                            for dc in range(DC):
                                nc.tensor.matmul(ps, ih_sb[:, dc, t_sl],
                                                 mlp_sb[:, dc, o_sl],
                                                 start=(dc == 0), stop=(dc == DC - 1))
                            nc.vector.tensor_copy(m_t[:, o_sl], ps)
                        nc.sync.dma_start(out=M_d[tc_], in_=m_t)

            # ================= Phase B: scores/softmax/att@U =================
            with tc.tile_pool(name="pb_att", bufs=2) as pb_att, \
                 tc.tile_pool(name="pb_one", bufs=1) as pb_one, \
                 tc.tile_pool(name="pb_str", bufs=2) as pb_str, \
                 tc.tile_pool(name="pb_st", bufs=2) as pb_st:
                attT_t = pb_one.tile([128, ST, 512], F16, name="attT", tag="attT")

                def scores_softmax(si):
                    st4 = si % 4
                    s_sl = slice(si * 128, si * 128 + 128)
                    scs = [
                        psum_pool.tile([128, 512], F32, name=f"sc{si}_{tb}",
                                       tag=f"sc{tb}")
                        for tb in range(TB)
                    ]
                    for ec in range(DC):
                        first = ec == 0
                        for tb in range(TB):
                            t_sl = slice(tb * 512, tb * 512 + 512)
                            nc.tensor.matmul(scs[tb], Ah_sb[:, ec, s_sl],
                                             ih_sb[:, ec, t_sl],
                                             start=first, stop=False)
                    for ec in range(DC):
                        last = ec == DC - 1
                        for tb in range(TB):
                            t_sl = slice(tb * 512, tb * 512 + 512)
                            nc.tensor.matmul(scs[tb], A8_sb[:, ec, :, s_sl],
                                             i8_sb[:, ec, :, t_sl],
                                             start=False, stop=last,
                                             perf_mode=DR)

                    # softmax: per-block max+exp, then algebraic rescale
                    st_t = pb_st.tile([128, 24], F32, name="st_t", tag="stats")
                    negm4 = st_t[:, 0:4]
                    sums = st_t[:, 4:8]
                    negM = st_t[:, 8:9]
                    S = st_t[:, 9:10]
                    recip = st_t[:, 10:11]
                    g4 = st_t[:, 12:16]
                    f4 = st_t[:, 16:20]
                    gs4 = st_t[:, 20:24]
                    att_t = pb_att.tile([128, LN], F16, name="att_t", tag="att")
                    for tb in range(TB):
                        t_sl = slice(tb * 512, tb * 512 + 512)
                        nc.vector.reduce_max(negm4[:, tb:tb + 1], scs[tb],
                                             axis=Axis.X, negate=True)
                        nc.scalar.activation(
                            out=att_t[:, t_sl], in_=scs[tb],
                            func=Act.Exp, bias=negm4[:, tb:tb + 1], scale=1.0,
                            accum_out=sums[:, tb:tb + 1],
                        )
                    nc.vector.tensor_reduce(negM, negm4, axis=Axis.X,
                                            op=mybir.AluOpType.min)
                    nc.scalar.activation(out=g4, in_=negm4, func=Act.Exp,
                                         bias=negM, scale=-1.0)
                    nc.vector.tensor_mul(gs4, g4, sums)
                    nc.vector.reduce_sum(S, gs4, axis=Axis.X)
                    nc.vector.reciprocal(recip, S)
                    nc.vector.tensor_scalar_mul(f4, g4, recip)

                    for tb in range(TB):
                        t_sl = slice(tb * 512, tb * 512 + 512)
                        nc.vector.tensor_scalar_mul(
                            att_t[:, t_sl], att_t[:, t_sl], f4[:, tb:tb + 1])
                    nc.sync.dma_start_transpose(
                        out=attT_t[:, :, st4 * 128:st4 * 128 + 128], in_=att_t)

                def av_out(si):
                    st4 = si % 4
                    s_sl = slice(si * 128, si * 128 + 128)
                    m_t = pb_str.tile([128, D], F16, name="m_t", tag="mst")
                    nc.gpsimd.dma_start(out=m_t, in_=M_d[si])
                    bias_t = pb_str.tile([128, D], F32, name="bias_t", tag="bias")
                    nc.gpsimd.dma_start(out=bias_t, in_=bias.ap()[s_sl, :])
                    out_t = pb_str.tile([128, D], F32, name="out_t", tag="out")
                    for ob in range(OB):
                        o_sl = slice(ob * 512, ob * 512 + 512)
                        ps = psum_pool.tile([128, 512], F32, name=f"av{si}_{ob}",
                                            tag=f"av{ob}")
                        for tc_ in range(ST):
                            nc.tensor.matmul(ps, attT_t[:, tc_, st4 * 128:st4 * 128 + 128],
                                             U_sb[:, tc_, o_sl],
                                             start=(tc_ == 0), stop=False)
                        nc.tensor.matmul(ps, id_sb, m_t[:, o_sl],
                                         start=False, stop=True)
                        nc.scalar.activation(
                            out=out_t[:, o_sl], in_=ps, func=Act.Prelu,
                            bias=0.0, scale=1.0, alpha=alpha_ap,
                        )
                    nc.vector.tensor_add(out_t, out_t, bias_t)
                    nc.gpsimd.dma_start(out=out_d.ap()[s_sl, :], in_=out_t)

                for g in range(4):
                    for st4 in range(4):
                        si = g * 4 + st4
                        scores_softmax(si)
                    for st4 in range(4):
                        si = g * 4 + st4
                        av_out(si)

            _psum_cm.__exit__(None, None, None)

    nc.compile()
    return nc


def _get_nc():
    global _cached_nc
    if _cached_nc is None:
        _cached_nc = _build()
    return _cached_nc


def _f16(x):
    return x.astype(np.float16)


def _e5(x):
    return x.astype(ml_dtypes.float8_e5m2)


def _prep_host(i, k, q, v, mlp, bias):
    W = (q.astype(np.float64) @ k.astype(np.float64).T).astype(np.float32)
    V2 = (v.astype(np.float64) @ mlp.astype(np.float64)).astype(np.float32)
    wh = _f16(W)
    wl32 = W - wh.astype(np.float32)
    w8 = np.stack([_e5(wl32), _e5(W)], axis=1)          # [D, 2, D]
    shared = dict(
        wh=wh, w8=w8, v2h=_f16(V2), mlph=_f16(mlp), bias=bias,
        ident=np.eye(128, dtype=np.float16),
    )
    in_maps = []
    for b in range(N_CORES):
        iT = np.ascontiguousarray(i[b].T)
        ih = _f16(iT)
        il32 = iT - ih.astype(np.float32)
        i8 = np.stack([_e5(iT), _e5(il32)], axis=1)     # [D, 2, LN]
        in_maps.append(dict(ih=ih, i8=i8, **shared))
    return in_maps


def kernel(i, k, q, v, mlp, bias):
    i = np.asarray(i, dtype=np.float32)
    k = np.asarray(k, dtype=np.float32)
    q = np.asarray(q, dtype=np.float32)
    v = np.asarray(v, dtype=np.float32)
    mlp = np.asarray(mlp, dtype=np.float32)
    bias = np.asarray(bias, dtype=np.float32)

    in_maps = _prep_host(i, k, q, v, mlp, bias)
    nc = _get_nc()
    res = bass_utils.run_bass_kernel_spmd(nc, in_maps, core_ids=list(range(N_CORES)))
    return np.stack([res.results[b]["out"] for b in range(N_CORES)])


# revision 5
# speedup vs baseline: 1.5949x; 1.0310x over previous
"""Trainium2 Bass kernel for nn_AttentionBlock (B=8, LN=2048, IDM=HDM=ODM=1024).

Sharding: data-parallel over batch, one batch element per NeuronCore (8 cores).

Math restructure (host precompute, fp64):
    W  = q @ k.T        so scores = (i@q) @ (i@k).T = i @ W @ i.T
    V2 = v @ mlp        so out    = lrelu(att @ (i@V2) + i@mlp) + bias
Per-core on-chip:
    A = i @ W           [ln, idm]   (transposed tiles: AT[e, s])
    scores = A @ i.T    [ln, ln]
    att = softmax(scores)
    U = i @ V2 ; M = i @ mlp
    out = lrelu(att @ U + M) + bias

Precision: the softmax amplifies score errors, so the QK path uses fp16
hi/lo pieces: main passes in fp16 (11-bit mantissa) plus the two cross
terms per matmul packed into fp8-e5m2 DoubleRow instructions (two K=128
contractions per instruction, ~2x rate). The value path (U/M/att@U) is
fp16 single-pass. All accumulation fp32 in PSUM.

Layout: contraction dim always on partitions. iT = i.T pieces come from
the host (ih fp16, i8 = [e5m2(i), e5m2(i - f16(i))] pair). A pieces are
derived on-chip from PSUM (Ah fp16 + A8 e5m2 pair). M is staged via DRAM
and re-added into the att@U PSUM group with an identity matmul.
"""
import numpy as np
import ml_dtypes

import concourse.bacc as bacc
import concourse.mybir as mybir
import concourse.tile as tile
from concourse import bass_utils

F32 = mybir.dt.float32
F16 = mybir.dt.float16
F8E5 = mybir.dt.float8e5
DR = mybir.MatmulPerfMode.DoubleRow
Act = mybir.ActivationFunctionType
Axis = mybir.AxisListType

LN = 2048      # sequence length
D = 1024       # idm = hdm = odm
N_CORES = 8
DC = D // 128      # 8 contraction chunks
ST = LN // 128     # 16 s-tiles
TB = LN // 512     # 4 t-blocks (N=512)
OB = D // 512      # 2 o-blocks
NEG_SLOPE = 0.2

_cached_nc = None


def _build():
    nc = bacc.Bacc("TRN2", target_bir_lowering=False, debug=False)

    ih = nc.dram_tensor("ih", [D, LN], F16, kind="ExternalInput")
    i8 = nc.dram_tensor("i8", [D, 2, LN], F8E5, kind="ExternalInput")
    wh = nc.dram_tensor("wh", [D, D], F16, kind="ExternalInput")
    w8 = nc.dram_tensor("w8", [D, 2, D], F8E5, kind="ExternalInput")
    v2h = nc.dram_tensor("v2h", [D, D], F16, kind="ExternalInput")
    mlph = nc.dram_tensor("mlph", [D, D], F16, kind="ExternalInput")
    bias = nc.dram_tensor("bias", [LN, D], F32, kind="ExternalInput")
    ident = nc.dram_tensor("ident", [128, 128], F16, kind="ExternalInput")
    out_d = nc.dram_tensor("out", [LN, D], F32, kind="ExternalOutput")

    # [D, X] viewed as [128 partitions, DC chunks, X]
    def pcv(t, x):
        return t.ap().rearrange("(c p) x -> p c x", p=128)

    def pcv2(t, x):  # [D, 2, X] -> [p, c, 2, X]
        return t.ap().rearrange("(c p) two x -> p c two x", p=128)

    ihv = pcv(ih, LN)
    i8v = pcv2(i8, LN)

    with tile.TileContext(nc) as tc:
        with tc.tile_pool(name="pers", bufs=1) as pers, \
             tc.tile_pool(name="dram", bufs=1, space="DRAM") as dram:
            ih_sb = pers.tile([128, DC, LN], F16)        # 32 KB/part
            i8_sb = pers.tile([128, DC, 2, LN], F8E5)    # 32 KB/part
            Ah_sb = pers.tile([128, DC, LN], F16)        # 32 KB/part
            A8_sb = pers.tile([128, DC, 2, LN], F8E5)    # 32 KB/part
            U_sb = pers.tile([128, ST, D], F16)          # 32 KB/part
            id_sb = pers.tile([128, 128], F16)
            alpha_ap = pers.tile([128, 1], F32)
            nc.vector.memset(alpha_ap, NEG_SLOPE)
            nc.sync.dma_start(out=id_sb, in_=ident.ap())

            M_d = dram.tile([ST, 128, D], F16)

            _psum_cm = tc.tile_pool(name="psum", bufs=1, space="PSUM")
            psum_pool = _psum_cm.__enter__()
            _ps_ctr = [0]

            def prep_psum(name):
                i_ = _ps_ctr[0] % 4
                _ps_ctr[0] += 1
                return psum_pool.tile([128, 512], F32, name=f"{name}{_ps_ctr[0]}",
                                      tag=f"sc{i_}")

            # ================= Phase A =================
            # Order: U (needs only ih+V2) runs first while w8/i8 stream in;
            # then A (wh streamed per-ec); then M (mlp reuses V2's slot).
            with tc.tile_pool(name="pa_w", bufs=1) as pa_w, \
                 tc.tile_pool(name="pa_whs", bufs=2) as pa_whs, \
                 tc.tile_pool(name="pa_m", bufs=2) as pa_m:
                v2_sb = pa_w.tile([128, DC, D], F16, name="v2_sb", tag="pav")
                w8_sb = pa_w.tile([128, DC, 2, D], F8E5, name="w8_sb", tag="pa8")
                for dc in range(DC):
                    nc.sync.dma_start(out=ih_sb[:, dc], in_=ihv[:, dc])
                for dc in range(DC):
                    nc.sync.dma_start(out=v2_sb[:, dc], in_=pcv(v2h, D)[:, dc])
                for dc in range(DC):
                    nc.sync.dma_start(out=w8_sb[:, dc], in_=pcv2(w8, D)[:, dc])
                    nc.sync.dma_start(out=i8_sb[:, dc], in_=i8v[:, dc])

                # --- U = i @ V2 -> [t, o] fp16 (stationary ih chunks)
                for tc_ in range(ST):
                    t_sl = slice(tc_ * 128, tc_ * 128 + 128)
                    for ob in range(OB):
                        ps = prep_psum("pu")
                        o_sl = slice(ob * 512, ob * 512 + 512)
                        for dc in range(DC):
                            nc.tensor.matmul(ps, ih_sb[:, dc, t_sl],
                                             v2_sb[:, dc, o_sl],
                                             start=(dc == 0), stop=(dc == DC - 1))
                        nc.vector.tensor_copy(U_sb[:, tc_, o_sl], ps)

                # --- A = i @ W  ->  AT[e, s] tiles, split to fp16 + e5m2 pair
                def a_group(wh_t, ec):
                    e_sl = slice(ec * 128, ec * 128 + 128)
                    for sb_ in range(TB):
                        ps = prep_psum("pa")
                        s_sl = slice(sb_ * 512, sb_ * 512 + 512)
                        for dc in range(DC):
                            nc.tensor.matmul(ps, wh_t[:, dc], ih_sb[:, dc, s_sl],
                                             start=(dc == 0), stop=False)
                        for dc in range(DC):
                            nc.tensor.matmul(ps, w8_sb[:, dc, :, e_sl],
                                             i8_sb[:, dc, :, s_sl],
                                             start=False, stop=(dc == DC - 1),
                                             perf_mode=DR)
                        nc.vector.tensor_copy(Ah_sb[:, ec, s_sl], ps)
                        nc.vector.tensor_sub(A8_sb[:, ec, 0, s_sl], ps,
                                             Ah_sb[:, ec, s_sl])
                        nc.vector.tensor_copy(A8_sb[:, ec, 1, s_sl], ps)

                pend = []
                for ec in range(DC):
                    wh_t = pa_whs.tile([128, DC, 128], F16, name="wh_t", tag="whs")
                    nc.sync.dma_start(out=wh_t, in_=pcv(wh, D)[:, :, ec * 128:ec * 128 + 128])
                    pend.append((wh_t, ec))
                    if len(pend) > 1:
                        a_group(*pend.pop(0))
                a_group(*pend.pop(0))

                # --- M = i @ mlp -> [s, o] fp16 -> DRAM staging
                mlp_sb = pa_w.tile([128, DC, D], F16, name="mlp_sb", tag="pav")
                for dc in range(DC):
                    nc.sync.dma_start(out=mlp_sb[:, dc], in_=pcv(mlph, D)[:, dc])
                for tc_ in range(ST):
                    t_sl = slice(tc_ * 128, tc_ * 128 + 128)
                    m_t = pa_m.tile([128, D], F16, name="m_t", tag="mst")
                    for ob in range(OB):
                        ps = prep_psum("pm")
                        o_sl = slice(ob * 512, ob * 512 + 512)
                        for dc in range(DC):
                            nc.tensor.matmul(ps, ih_sb[:, dc, t_sl],
                                             mlp_sb[:, dc, o_sl],
                                             start=(dc == 0), stop=(dc == DC - 1))
                        nc.vector.tensor_copy(m_t[:, o_sl], ps)
                    nc.sync.dma_start(out=M_d[tc_], in_=m_t)

            # ================= Phase B: scores/softmax/att@U =================
            with tc.tile_pool(name="pb_att", bufs=2) as pb_att, \
                 tc.tile_pool(name="pb_one", bufs=1) as pb_one, \
                 tc.tile_pool(name="pb_str", bufs=2) as pb_str, \
                 tc.tile_pool(name="pb_st", bufs=2) as pb_st:
                attT_t = pb_one.tile([128, ST, 512], F16, name="attT", tag="attT")

                def scores_softmax(si):
                    st4 = si % 4
                    s_sl = slice(si * 128, si * 128 + 128)
                    scs = [
                        psum_pool.tile([128, 512], F32, name=f"sc{si}_{tb}",
                                       tag=f"sc{tb}")
                        for tb in range(TB)
                    ]
                    for ec in range(DC):
                        first = ec == 0
                        for tb in range(TB):
                            t_sl = slice(tb * 512, tb * 512 + 512)
                            nc.tensor.matmul(scs[tb], Ah_sb[:, ec, s_sl],
                                             ih_sb[:, ec, t_sl],
                                             start=first, stop=False)
                    for ec in range(DC):
                        last = ec == DC - 1
                        for tb in range(TB):
                            t_sl = slice(tb * 512, tb * 512 + 512)
                            nc.tensor.matmul(scs[tb], A8_sb[:, ec, :, s_sl],
                                             i8_sb[:, ec, :, t_sl],
                                             start=False, stop=last,
                                             perf_mode=DR)

                    # softmax: per-block max+exp, then algebraic rescale
                    st_t = pb_st.tile([128, 24], F32, name="st_t", tag="stats")
                    negm4 = st_t[:, 0:4]
                    sums = st_t[:, 4:8]
                    negM = st_t[:, 8:9]
                    S = st_t[:, 9:10]
                    recip = st_t[:, 10:11]
                    g4 = st_t[:, 12:16]
                    f4 = st_t[:, 16:20]
                    gs4 = st_t[:, 20:24]
                    att_t = pb_att.tile([128, LN], F16, name="att_t", tag="att")
                    for tb in range(TB):
                        t_sl = slice(tb * 512, tb * 512 + 512)
                        nc.vector.reduce_max(negm4[:, tb:tb + 1], scs[tb],
                                             axis=Axis.X, negate=True)
                        nc.scalar.activation(
                            out=att_t[:, t_sl], in_=scs[tb],
                            func=Act.Exp, bias=negm4[:, tb:tb + 1], scale=1.0,
                            accum_out=sums[:, tb:tb + 1],
                        )
                    nc.vector.tensor_reduce(negM, negm4, axis=Axis.X,
                                            op=mybir.AluOpType.min)
                    nc.scalar.activation(out=g4, in_=negm4, func=Act.Exp,
                                         bias=negM, scale=-1.0)
                    nc.vector.tensor_mul(gs4, g4, sums)
                    nc.vector.reduce_sum(S, gs4, axis=Axis.X)
                    nc.vector.reciprocal(recip, S)
                    nc.vector.tensor_scalar_mul(f4, g4, recip)

                    for tb in range(TB):
                        t_sl = slice(tb * 512, tb * 512 + 512)
                        nc.vector.tensor_scalar_mul(
                            att_t[:, t_sl], att_t[:, t_sl], f4[:, tb:tb + 1])
                    nc.sync.dma_start_transpose(
                        out=attT_t[:, :, st4 * 128:st4 * 128 + 128], in_=att_t)

                def av_out(si):
                    st4 = si % 4
                    s_sl = slice(si * 128, si * 128 + 128)
                    m_t = pb_str.tile([128, D], F16, name="m_t", tag="mst")
                    nc.gpsimd.dma_start(out=m_t, in_=M_d[si])
                    bias_t = pb_str.tile([128, D], F32, name="bias_t", tag="bias")
                    nc.gpsimd.dma_start(out=bias_t, in_=bias.ap()[s_sl, :])
                    out_t = pb_str.tile([128, D], F32, name="out_t", tag="out")
                    for ob in range(OB):
                        o_sl = slice(ob * 512, ob * 512 + 512)
                        ps = psum_pool.tile([128, 512], F32, name=f"av{si}_{ob}",
                                            tag=f"av{ob}")
                        for tc_ in range(ST):
                            nc.tensor.matmul(ps, attT_t[:, tc_, st4 * 128:st4 * 128 + 128],
                                             U_sb[:, tc_, o_sl],
                                             start=(tc_ == 0), stop=False)
                        nc.tensor.matmul(ps, id_sb, m_t[:, o_sl],
                                         start=False, stop=True)
                        nc.scalar.activation(
                            out=out_t[:, o_sl], in_=ps, func=Act.Prelu,
                            bias=0.0, scale=1.0, alpha=alpha_ap,
                        )
                    nc.vector.tensor_add(out_t, out_t, bias_t)
                    nc.gpsimd.dma_start(out=out_d.ap()[s_sl, :], in_=out_t)

                for g in range(4):
                    for st4 in range(4):
                        si = g * 4 + st4
                        scores_softmax(si)
                    for st4 in range(4):
                        si = g * 4 + st4
                        av_out(si)

            _psum_cm.__exit__(None, None, None)

    nc.compile()
    return nc


def _get_nc():
    global _cached_nc
    if _cached_nc is None:
        _cached_nc = _build()
    return _cached_nc


def _f16(x):
    return x.astype(np.float16)


def _e5(x):
    return x.astype(ml_dtypes.float8_e5m2)


def _prep_host(i, k, q, v, mlp, bias):
    W = (q.astype(np.float64) @ k.astype(np.float64).T).astype(np.float32)
    V2 = (v.astype(np.float64) @ mlp.astype(np.float64)).astype(np.float32)
    wh = _f16(W)
    wl32 = W - wh.astype(np.float32)
    w8 = np.stack([_e5(wl32), _e5(W)], axis=1)          # [D, 2, D]
    shared = dict(
        wh=wh, w8=w8, v2h=_f16(V2), mlph=_f16(mlp), bias=bias,
        ident=np.eye(128, dtype=np.float16),
    )
    in_maps = []
    for b in range(N_CORES):
        iT = np.ascontiguousarray(i[b].T)
        ih = _f16(iT)
        il32 = iT - ih.astype(np.float32)
        i8 = np.stack([_e5(iT), _e5(il32)], axis=1)     # [D, 2, LN]
        in_maps.append(dict(ih=ih, i8=i8, **shared))
    return in_maps


def kernel(i, k, q, v, mlp, bias):
    i = np.asarray(i, dtype=np.float32)
    k = np.asarray(k, dtype=np.float32)
    q = np.asarray(q, dtype=np.float32)
    v = np.asarray(v, dtype=np.float32)
    mlp = np.asarray(mlp, dtype=np.float32)
    bias = np.asarray(bias, dtype=np.float32)

    in_maps = _prep_host(i, k, q, v, mlp, bias)
    nc = _get_nc()
    res = bass_utils.run_bass_kernel_spmd(nc, in_maps, core_ids=list(range(N_CORES)))
    return np.stack([res.results[b]["out"] for b in range(N_CORES)])


# revision 6
# speedup vs baseline: 1.6180x; 1.0145x over previous
"""Trainium2 Bass kernel for nn_AttentionBlock (B=8, LN=2048, IDM=HDM=ODM=1024).

Sharding: data-parallel over batch, one batch element per NeuronCore (8 cores).

Math restructure (host precompute, fp64):
    W  = q @ k.T        so scores = (i@q) @ (i@k).T = i @ W @ i.T
    V2 = v @ mlp        so out    = lrelu(att @ (i@V2) + i@mlp) + bias
Per-core on-chip:
    A = i @ W           [ln, idm]   (transposed tiles: AT[e, s])
    scores = A @ i.T    [ln, ln]
    att = softmax(scores)
    U = i @ V2 ; M = i @ mlp
    out = lrelu(att @ U + M) + bias

Precision: the softmax amplifies score errors, so the QK path uses fp16
hi/lo pieces: main passes in fp16 (11-bit mantissa) plus the two cross
terms per matmul packed into fp8-e5m2 DoubleRow instructions (two K=128
contractions per instruction, ~2x rate). The value path (U/M/att@U) is
fp16 single-pass. All accumulation fp32 in PSUM.

Layout: contraction dim always on partitions. iT = i.T pieces come from
the host (ih fp16, i8 = [e5m2(i), e5m2(i - f16(i))] pair). A pieces are
derived on-chip from PSUM (Ah fp16 + A8 e5m2 pair). M is staged via DRAM
and re-added into the att@U PSUM group with an identity matmul.
"""
import numpy as np
import ml_dtypes

import concourse.bacc as bacc
import concourse.mybir as mybir
import concourse.tile as tile
from concourse import bass_utils

F32 = mybir.dt.float32
F16 = mybir.dt.float16
F8E5 = mybir.dt.float8e5
DR = mybir.MatmulPerfMode.DoubleRow
Act = mybir.ActivationFunctionType
Axis = mybir.AxisListType

LN = 2048      # sequence length
D = 1024       # idm = hdm = odm
N_CORES = 8
DC = D // 128      # 8 contraction chunks
ST = LN // 128     # 16 s-tiles
TB = LN // 512     # 4 t-blocks (N=512)
OB = D // 512      # 2 o-blocks
NEG_SLOPE = 0.2

_cached_nc = None


def _build():
    nc = bacc.Bacc("TRN2", target_bir_lowering=False, debug=False)

    ih = nc.dram_tensor("ih", [D, LN], F16, kind="ExternalInput")
    i8 = nc.dram_tensor("i8", [D, 2, LN], F8E5, kind="ExternalInput")
    wh = nc.dram_tensor("wh", [D, D], F16, kind="ExternalInput")
    w8 = nc.dram_tensor("w8", [D, 2, D], F8E5, kind="ExternalInput")
    v2h = nc.dram_tensor("v2h", [D, D], F16, kind="ExternalInput")
    mlph = nc.dram_tensor("mlph", [D, D], F16, kind="ExternalInput")
    bias = nc.dram_tensor("bias", [LN, D], F32, kind="ExternalInput")
    ident = nc.dram_tensor("ident", [128, 128], F16, kind="ExternalInput")
    out_d = nc.dram_tensor("out", [LN, D], F32, kind="ExternalOutput")

    # [D, X] viewed as [128 partitions, DC chunks, X]
    def pcv(t, x):
        return t.ap().rearrange("(c p) x -> p c x", p=128)

    def pcv2(t, x):  # [D, 2, X] -> [p, c, 2, X]
        return t.ap().rearrange("(c p) two x -> p c two x", p=128)

    ihv = pcv(ih, LN)
    i8v = pcv2(i8, LN)

    with tile.TileContext(nc) as tc:
        with tc.tile_pool(name="pers", bufs=1) as pers, \
             tc.tile_pool(name="dram", bufs=1, space="DRAM") as dram:
            ih_sb = pers.tile([128, DC, LN], F16)        # 32 KB/part
            i8_sb = pers.tile([128, DC, 2, LN], F8E5)    # 32 KB/part
            Ah_sb = pers.tile([128, DC, LN], F16)        # 32 KB/part
            A8_sb = pers.tile([128, DC, 2, LN], F8E5)    # 32 KB/part
            U_sb = pers.tile([128, ST, D], F16)          # 32 KB/part
            id_sb = pers.tile([128, 128], F16)
            alpha_ap = pers.tile([128, 1], F32)
            nc.vector.memset(alpha_ap, NEG_SLOPE)
            nc.sync.dma_start(out=id_sb, in_=ident.ap())

            M_d = dram.tile([ST, 128, D], F16)

            _psum_cm = tc.tile_pool(name="psum", bufs=1, space="PSUM")
            psum_pool = _psum_cm.__enter__()
            _ps_ctr = [0]
            _ps_tags = ["sc0", "sc1", "sc2", "sc3", "av0", "av1"]

            def prep_psum(name):
                tag = _ps_tags[_ps_ctr[0] % 6]
                _ps_ctr[0] += 1
                return psum_pool.tile([128, 512], F32, name=f"{name}{_ps_ctr[0]}",
                                      tag=tag)

            # ================= Phase A =================
            # Order: U (needs only ih+V2) runs first while w8/i8 stream in;
            # then A (wh streamed per-ec); then M (mlp reuses V2's slot).
            with tc.tile_pool(name="pa_w", bufs=1) as pa_w, \
                 tc.tile_pool(name="pa_whs", bufs=2) as pa_whs, \
                 tc.tile_pool(name="pa_m", bufs=2) as pa_m:
                v2_sb = pa_w.tile([128, DC, D], F16, name="v2_sb", tag="pav")
                w8_sb = pa_w.tile([128, DC, 2, D], F8E5, name="w8_sb", tag="pa8")
                v2v = pcv(v2h, D)
                nc.sync.dma_start(out=ih_sb[:, :, 0:512], in_=ihv[:, :, 0:512])
                nc.sync.dma_start(out=v2_sb[:, :, 0:512], in_=v2v[:, :, 0:512])
                for cb in range(1, 4):
                    c_sl = slice(cb * 512, cb * 512 + 512)
                    nc.sync.dma_start(out=ih_sb[:, :, c_sl], in_=ihv[:, :, c_sl])
                nc.sync.dma_start(out=v2_sb[:, :, 512:1024], in_=v2v[:, :, 512:1024])
                for dc in range(DC):
                    nc.sync.dma_start(out=w8_sb[:, dc], in_=pcv2(w8, D)[:, dc])
                    nc.sync.dma_start(out=i8_sb[:, dc], in_=i8v[:, dc])

                # --- U = i @ V2 -> [t, o] fp16 (stationary ih chunks)
                for ob in range(OB):
                    o_sl = slice(ob * 512, ob * 512 + 512)
                    for tc_ in range(ST):
                        t_sl = slice(tc_ * 128, tc_ * 128 + 128)
                        ps = prep_psum("pu")
                        for dc in range(DC):
                            nc.tensor.matmul(ps, ih_sb[:, dc, t_sl],
                                             v2_sb[:, dc, o_sl],
                                             start=(dc == 0), stop=(dc == DC - 1))
                        nc.vector.tensor_copy(U_sb[:, tc_, o_sl], ps)

                # --- A = i @ W  ->  AT[e, s] tiles, split to fp16 + e5m2 pair
                def a_group(wh_t, ec):
                    e_sl = slice(ec * 128, ec * 128 + 128)
                    for sb_ in range(TB):
                        ps = prep_psum("pa")
                        s_sl = slice(sb_ * 512, sb_ * 512 + 512)
                        for dc in range(DC):
                            nc.tensor.matmul(ps, wh_t[:, dc], ih_sb[:, dc, s_sl],
                                             start=(dc == 0), stop=False)
                        for dc in range(DC):
                            nc.tensor.matmul(ps, w8_sb[:, dc, :, e_sl],
                                             i8_sb[:, dc, :, s_sl],
                                             start=False, stop=(dc == DC - 1),
                                             perf_mode=DR)
                        nc.vector.tensor_copy(Ah_sb[:, ec, s_sl], ps)
                        nc.vector.tensor_sub(A8_sb[:, ec, 0, s_sl], ps,
                                             Ah_sb[:, ec, s_sl])
                        nc.vector.tensor_copy(A8_sb[:, ec, 1, s_sl], ps)

                pend = []
                for ec in range(DC):
                    wh_t = pa_whs.tile([128, DC, 128], F16, name="wh_t", tag="whs")
                    nc.sync.dma_start(out=wh_t, in_=pcv(wh, D)[:, :, ec * 128:ec * 128 + 128])
                    pend.append((wh_t, ec))
                    if len(pend) > 1:
                        a_group(*pend.pop(0))
                a_group(*pend.pop(0))

                # --- M = i @ mlp -> [s, o] fp16 -> DRAM staging
                mlp_sb = pa_w.tile([128, DC, D], F16, name="mlp_sb", tag="pav")
                for dc in range(DC):
                    nc.sync.dma_start(out=mlp_sb[:, dc], in_=pcv(mlph, D)[:, dc])
                for tc_ in range(ST):
                    t_sl = slice(tc_ * 128, tc_ * 128 + 128)
                    m_t = pa_m.tile([128, D], F16, name="m_t", tag="mst")
                    for ob in range(OB):
                        ps = prep_psum("pm")
                        o_sl = slice(ob * 512, ob * 512 + 512)
                        for dc in range(DC):
                            nc.tensor.matmul(ps, ih_sb[:, dc, t_sl],
                                             mlp_sb[:, dc, o_sl],
                                             start=(dc == 0), stop=(dc == DC - 1))
                        nc.vector.tensor_copy(m_t[:, o_sl], ps)
                    nc.sync.dma_start(out=M_d[tc_], in_=m_t)

            # ================= Phase B: scores/softmax/att@U =================
            with tc.tile_pool(name="pb_att", bufs=2) as pb_att, \
                 tc.tile_pool(name="pb_one", bufs=1) as pb_one, \
                 tc.tile_pool(name="pb_str", bufs=2) as pb_str, \
                 tc.tile_pool(name="pb_st", bufs=2) as pb_st:
                attT_t = pb_one.tile([128, ST, 512], F16, name="attT", tag="attT")

                def scores_softmax(si):
                    st4 = si % 4
                    s_sl = slice(si * 128, si * 128 + 128)
                    scs = [
                        psum_pool.tile([128, 512], F32, name=f"sc{si}_{tb}",
                                       tag=f"sc{tb}")
                        for tb in range(TB)
                    ]
                    for ec in range(DC):
                        first = ec == 0
                        for tb in range(TB):
                            t_sl = slice(tb * 512, tb * 512 + 512)
                            nc.tensor.matmul(scs[tb], Ah_sb[:, ec, s_sl],
                                             ih_sb[:, ec, t_sl],
                                             start=first, stop=False)
                    for ec in range(DC):
                        last = ec == DC - 1
                        for tb in range(TB):
                            t_sl = slice(tb * 512, tb * 512 + 512)
                            nc.tensor.matmul(scs[tb], A8_sb[:, ec, :, s_sl],
                                             i8_sb[:, ec, :, t_sl],
                                             start=False, stop=last,
                                             perf_mode=DR)

                    # softmax: per-block max+exp, then algebraic rescale
                    st_t = pb_st.tile([128, 24], F32, name="st_t", tag="stats")
                    negm4 = st_t[:, 0:4]
                    sums = st_t[:, 4:8]
                    negM = st_t[:, 8:9]
                    S = st_t[:, 9:10]
                    recip = st_t[:, 10:11]
                    g4 = st_t[:, 12:16]
                    f4 = st_t[:, 16:20]
                    gs4 = st_t[:, 20:24]
                    att_t = pb_att.tile([128, LN], F16, name="att_t", tag="att")
                    for tb in range(TB):
                        t_sl = slice(tb * 512, tb * 512 + 512)
                        nc.vector.reduce_max(negm4[:, tb:tb + 1], scs[tb],
                                             axis=Axis.X, negate=True)
                        nc.scalar.activation(
                            out=att_t[:, t_sl], in_=scs[tb],
                            func=Act.Exp, bias=negm4[:, tb:tb + 1], scale=1.0,
                            accum_out=sums[:, tb:tb + 1],
                        )
                    nc.vector.tensor_reduce(negM, negm4, axis=Axis.X,
                                            op=mybir.AluOpType.min)
                    nc.scalar.activation(out=g4, in_=negm4, func=Act.Exp,
                                         bias=negM, scale=-1.0)
                    nc.vector.tensor_mul(gs4, g4, sums)
                    nc.vector.reduce_sum(S, gs4, axis=Axis.X)
                    nc.vector.reciprocal(recip, S)
                    nc.vector.tensor_scalar_mul(f4, g4, recip)

                    for tb in range(TB):
                        t_sl = slice(tb * 512, tb * 512 + 512)
                        nc.vector.tensor_scalar_mul(
                            att_t[:, t_sl], att_t[:, t_sl], f4[:, tb:tb + 1])
                    nc.sync.dma_start_transpose(
                        out=attT_t[:, :, st4 * 128:st4 * 128 + 128], in_=att_t)

                def av_out(si):
                    st4 = si % 4
                    s_sl = slice(si * 128, si * 128 + 128)
                    m_t = pb_str.tile([128, D], F16, name="m_t", tag="mst")
                    nc.gpsimd.dma_start(out=m_t, in_=M_d[si])
                    bias_t = pb_str.tile([128, D], F32, name="bias_t", tag="bias")
                    nc.gpsimd.dma_start(out=bias_t, in_=bias.ap()[s_sl, :])
                    out_t = pb_str.tile([128, D], F32, name="out_t", tag="out")
                    for ob in range(OB):
                        o_sl = slice(ob * 512, ob * 512 + 512)
                        ps = psum_pool.tile([128, 512], F32, name=f"av{si}_{ob}",
                                            tag=f"av{ob}")
                        for tc_ in range(ST):
                            nc.tensor.matmul(ps, attT_t[:, tc_, st4 * 128:st4 * 128 + 128],
                                             U_sb[:, tc_, o_sl],
                                             start=(tc_ == 0), stop=False)
                        nc.tensor.matmul(ps, id_sb, m_t[:, o_sl],
                                         start=False, stop=True)
                        nc.scalar.activation(
                            out=out_t[:, o_sl], in_=ps, func=Act.Prelu,
                            bias=0.0, scale=1.0, alpha=alpha_ap,
                        )
                    nc.vector.tensor_add(out_t, out_t, bias_t)
                    nc.gpsimd.dma_start(out=out_d.ap()[s_sl, :], in_=out_t)

                for si in range(ST):
                    scores_softmax(si)
                    if si > 0:
                        av_out(si - 1)
                av_out(ST - 1)

            _psum_cm.__exit__(None, None, None)

    nc.compile()
    return nc


def _get_nc():
    global _cached_nc
    if _cached_nc is None:
        _cached_nc = _build()
    return _cached_nc


def _f16(x):
    return x.astype(np.float16)


def _e5(x):
    return x.astype(ml_dtypes.float8_e5m2)


def _prep_host(i, k, q, v, mlp, bias):
    W = (q.astype(np.float64) @ k.astype(np.float64).T).astype(np.float32)
    V2 = (v.astype(np.float64) @ mlp.astype(np.float64)).astype(np.float32)
    wh = _f16(W)
    wl32 = W - wh.astype(np.float32)
    w8 = np.stack([_e5(wl32), _e5(W)], axis=1)          # [D, 2, D]
    shared = dict(
        wh=wh, w8=w8, v2h=_f16(V2), mlph=_f16(mlp), bias=bias,
        ident=np.eye(128, dtype=np.float16),
    )
    in_maps = []
    for b in range(N_CORES):
        iT = np.ascontiguousarray(i[b].T)
        ih = _f16(iT)
        il32 = iT - ih.astype(np.float32)
        i8 = np.stack([_e5(iT), _e5(il32)], axis=1)     # [D, 2, LN]
        in_maps.append(dict(ih=ih, i8=i8, **shared))
    return in_maps


def kernel(i, k, q, v, mlp, bias):
    i = np.asarray(i, dtype=np.float32)
    k = np.asarray(k, dtype=np.float32)
    q = np.asarray(q, dtype=np.float32)
    v = np.asarray(v, dtype=np.float32)
    mlp = np.asarray(mlp, dtype=np.float32)
    bias = np.asarray(bias, dtype=np.float32)

    in_maps = _prep_host(i, k, q, v, mlp, bias)
    nc = _get_nc()
    res = bass_utils.run_bass_kernel_spmd(nc, in_maps, core_ids=list(range(N_CORES)))
    return np.stack([res.results[b]["out"] for b in range(N_CORES)])
